# revision 52
# baseline (speedup 1.0000x reference)
"""MemoryCrossAttention Trainium2 Bass kernel (fp16 full-rate rewrite).

8-core data-parallel over query rows: core c handles batch c//2, row-half
c%2 (2048 rows). All matmuls run in fp16 (full PE rate; fp32r is
throttled to half rate on TRN2 hardware). Weights and x are cast to fp16
on the host with p-major tiling so every weight group loads in one
large-line DMA. Everything stays resident in SBUF (no DRAM spills).
Emission order keeps the PE fed: K/V projections run first (they only
need the small memory-token tiles), the RMSNorm square-sum accumulates
behind them, and phase C is software-pipelined (Q-projection of head h+1
is emitted between the scores and denominator of head h so the exp/
reciprocal latency never stalls the PE). The gate projection is fused
into the O-projection with sigmoid applied at PSUM eviction.
"""
from concourse.bass_utils import run_bass_kernel_spmd


from contextlib import ExitStack

import concourse.bass as bass
import concourse.tile as tile
from concourse import mybir

F32 = mybir.dt.float32
F16 = mybir.dt.float16
P = 128
LN256 = 5.545177444479562


def _bcast_ap(row_ap):
    """[1, n] AP -> [128, n] partition-broadcast AP (stride 0)."""
    return bass.AP(tensor=row_ap.tensor, offset=row_ap.offset,
                   ap=[[0, P]] + row_ap.ap)


def build(nc, H, NH, R, M, eps=1e-6):
    HD = 128
    assert H == NH * HD
    KT = H // P           # contraction tiles (16)
    LQ = R // 512         # 512-wide row chunks (4)
    MT = M // P           # memory-token partition tiles (2)
    NHG = NH // 2         # 2-head weight groups (8)
    KH = 8                # heads per K-proj psum group
    NKG = NH // KH        # 2
    NVC = H // 512        # V d-chunks (4)
    scale = HD ** -0.5

    xhT = nc.dram_tensor("xhT", [P, KT, R], F16, kind="ExternalInput")
    memT = nc.dram_tensor("memT", [P, KT, M], F16, kind="ExternalInput")
    maskb = nc.dram_tensor("maskb", [P, MT], F32, kind="ExternalInput")
    wqT = nc.dram_tensor("wqT", [NHG, P, KT, 2 * P], F16, kind="ExternalInput")
    wgT = nc.dram_tensor("wgT", [NHG, P, KT, 2 * P], F16, kind="ExternalInput")
    woT = nc.dram_tensor("woT", [NHG, P, KT, 2 * P], F16, kind="ExternalInput")
    wkT = nc.dram_tensor("wkT", [NKG, P, KT, KH * P], F16, kind="ExternalInput")
    wvT = nc.dram_tensor("wvT", [NVC, P, KT, 512], F16, kind="ExternalInput")
    outT = nc.dram_tensor("outT", [H, R], F32, kind="ExternalOutput")

    with tile.TileContext(nc) as tc, ExitStack() as ctx:
        dram = ctx.enter_context(tc.tile_pool(name="dram", bufs=1, space="DRAM"))
        rs_scr = dram.tile([LQ, 512], F16)
        rd_scr = dram.tile([NH, LQ, 512], F16)

        const = ctx.enter_context(tc.tile_pool(name="const", bufs=1))
        ones_t = const.tile([P, 1], F16)
        nc.vector.memset(ones_t, 1.0)
        eps_sb = const.tile([P, 1], F32)
        nc.vector.memset(eps_sb, eps)
        mask_sb = const.tile([P, MT], F32)
        nc.sync.dma_start(out=mask_sb, in_=maskb[:])

        # persistent activations
        xpool = ctx.enter_context(tc.tile_pool(name="x", bufs=1))
        xh = xpool.tile([P, KT, R], F16)          # x, then xn in place
        kvpool = ctx.enter_context(tc.tile_pool(name="kv", bufs=1))
        kT_all = kvpool.tile([P, NH, M], F16)     # [d, h, m]
        vmd = kvpool.tile([P, MT, H], F16)        # [m, mt, d_full]
        apool = ctx.enter_context(tc.tile_pool(name="attn", bufs=1))
        attn_all = apool.tile([P, NH, R], F16)    # [d, h, rows]

        # ===== Phase A+B: rmsnorm first (paced by x DMA), then K/V =====
        # Queue split: gpsimd carries x + V weights, scalar carries memory
        # tokens + K weights, sync carries the small rs/rden round-trips
        # and the Q/O/gate weight groups. The square-sum matmuls run while
        # x streams in; K/V matmuls fill the PE right after, overlapping
        # the rs reciprocal chain and the xn normalize on vector.
        with tc.tile_pool(name="memp", bufs=1) as memp:
            memh = memp.tile([P, KT, M], F16)
            for ka, kb in ((0, 2), (2, 4), (4, 8), (8, 12), (12, 16)):
                ks = slice(ka, kb)
                nc.sync.dma_start(out=xh[:, ks, :], in_=xhT[:, ks, :])
            nc.scalar.dma_start(out=memh, in_=memT[:])

            with tc.tile_pool(name="wkst", bufs=3) as wkst:
                KT4 = KT // 4
                wk_t = {}
                for q in range(8):
                    kg, qq = divmod(q, 4)
                    wk_t[kg, qq] = wkst.tile([P, KT4, KH * P], F16,
                                             name="wk_t")
                    nc.scalar.dma_start(
                        out=wk_t[kg, qq],
                        in_=wkT[kg][:, qq * KT4:(qq + 1) * KT4, :])

                with tc.tile_pool(name="x2", bufs=2) as x2p, \
                     tc.tile_pool(name="ssqp", bufs=1, space="PSUM") as ssqp, \
                     tc.tile_pool(name="rsp", bufs=1) as rsp, \
                     tc.tile_pool(name="rsrow", bufs=2) as rsrowp:
                    ssq = [ssqp.tile([1, 512], F32, name=f"ssq{j}")
                           for j in range(LQ)]
                    for kt in range(KT):
                        x2 = x2p.tile([P, R], F16)
                        nc.vector.tensor_mul(x2, xh[:, kt, :], xh[:, kt, :])
                        for lq in range(LQ):
                            nc.tensor.matmul(
                                ssq[lq], ones_t,
                                x2[:, lq * 512:(lq + 1) * 512],
                                start=(kt == 0), stop=(kt == KT - 1))
                    rsb = rsp.tile([P, R], F16)
                    for lq in range(LQ):
                        srow = rsrowp.tile([1, 512], F32, name="srow")
                        rsrow = rsrowp.tile([1, 512], F32, name="rsrow")
                        rshrow = rsrowp.tile([1, 512], F16, name="rshrow")
                        nc.scalar.activation(
                            srow, ssq[lq],
                            mybir.ActivationFunctionType.Sqrt,
                            bias=eps_sb[0:1, :], scale=1.0 / H)
                        nc.vector.reciprocal_approx_fast(rsrow, srow)
                        nc.vector.tensor_copy(rshrow, rsrow)
                        nc.sync.dma_start(out=rs_scr[lq, :], in_=rshrow)
                        nc.sync.dma_start(
                            out=rsb[:, lq * 512:(lq + 1) * 512],
                            in_=_bcast_ap(rs_scr[lq, :]))
                    # normalize x in place (overlaps K/V matmuls on PE)
                    for lq in range(LQ):
                        c = slice(lq * 512, (lq + 1) * 512)
                        for kt in range(KT):
                            nc.vector.tensor_mul(xh[:, kt, c], xh[:, kt, c],
                                                 rsb[:, c])



                # K projection (PE fills right after the square-sums)
                with tc.tile_pool(name="kps", bufs=1, space="PSUM") as kps:
                    for kg in range(NKG):
                        kpsum = [kps.tile([P, M], F32, name=f"kpsum{i}")
                                 for i in range(KH)]
                        for qq in range(4):
                            for k4 in range(KT4):
                                kt = qq * KT4 + k4
                                for hh in range(KH):
                                    nc.tensor.matmul(
                                        kpsum[hh],
                                        wk_t[kg, qq][:, k4,
                                                     hh * P:(hh + 1) * P],
                                        memh[:, kt, :],
                                        start=(kt == 0),
                                        stop=(kt == KT - 1))
                        for hh in range(KH):
                            if kg == 0 or hh % 2 == 0:
                                nc.scalar.copy(kT_all[:, kg * KH + hh, :],
                                               kpsum[hh])
                            else:
                                nc.vector.tensor_copy(
                                    kT_all[:, kg * KH + hh, :], kpsum[hh])
            with tc.tile_pool(name="wvst", bufs=3) as wvst, \
                 tc.tile_pool(name="vps", bufs=2, space="PSUM") as vps:
                for dc in range(NVC):
                    wv_t = wvst.tile([P, KT, 512], F16)
                    nc.sync.dma_start(out=wv_t, in_=wvT[dc])
                    vpsum = [vps.tile([P, 512], F32, name=f"vpsum{i}")
                             for i in range(MT)]
                    for kt in range(KT):
                        for mt in range(MT):
                            nc.tensor.matmul(
                                vpsum[mt], memh[:, kt, mt * P:(mt + 1) * P],
                                wv_t[:, kt, :],
                                start=(kt == 0), stop=(kt == KT - 1))
                    for mt in range(MT):
                        if dc < NVC - 1 or mt == 0:
                            nc.scalar.copy(
                                vmd[:, mt, dc * 512:(dc + 1) * 512],
                                vpsum[mt])
                        else:
                            nc.vector.tensor_copy(
                                vmd[:, mt, dc * 512:(dc + 1) * 512],
                                vpsum[mt])

        # ====== preload O/gate group 0 so phase D starts instantly ======
        with tc.tile_pool(name="wod0", bufs=1) as w0p:
            wo0 = w0p.tile([P, KT, 2 * P], F16, name="wo0")
            wg0 = w0p.tile([P, KT, 2 * P], F16, name="wg0")

            # ===== Phase C: Q proj + attention, software-pipelined =====
            with tc.tile_pool(name="wqst", bufs=2) as wqst, \
                 tc.tile_pool(name="qh", bufs=2) as qhp, \
                 tc.tile_pool(name="probs", bufs=1) as probsp, \
                 tc.tile_pool(name="rden", bufs=1) as rdenp, \
                 tc.tile_pool(name="rrow", bufs=2) as rrowp, \
                 tc.tile_pool(name="qps", bufs=2, space="PSUM") as qps, \
                 tc.tile_pool(name="dpps", bufs=2, space="PSUM") as dpps, \
                 tc.tile_pool(name="tmpps", bufs=2, space="PSUM") as tmpps:
                wq_tiles = {}

                def qproj(h):
                    hg, hh = divmod(h, 2)
                    if hh == 0:
                        wq_t = wqst.tile([P, KT, 2 * P], F16, name="wq_t")
                        nc.sync.dma_start(out=wq_t, in_=wqT[hg])
                        wq_tiles[hg] = wq_t
                    wq_t = wq_tiles[hg]
                    qh = qhp.tile([P, R], F16, name="qh")
                    for lq in range(LQ):
                        qpsum = qps.tile([P, 512], F32, name="qpsum")
                        for kt in range(KT):
                            nc.tensor.matmul(
                                qpsum, wq_t[:, kt, hh * P:(hh + 1) * P],
                                xh[:, kt, lq * 512:(lq + 1) * 512],
                                start=(kt == 0), stop=(kt == KT - 1))
                        nc.scalar.copy(qh[:, lq * 512:(lq + 1) * 512], qpsum)
                    return qh

                qh_cur = qproj(0)
                for h in range(NH):
                    last = h == NH - 1
                    probs = probsp.tile([P, MT, R], F16, name="probs")
                    rdenb = rdenp.tile([P, R], F16, name="rdenb")

                    def scores(lqs):
                        # scores -> probs (exp, mask bias, /256 folded in)
                        for mt in range(MT):
                            for lq in lqs:
                                sp = tmpps.tile([P, 512], F32, name="sp")
                                nc.tensor.matmul(
                                    sp, kT_all[:, h, mt * P:(mt + 1) * P],
                                    qh_cur[:, lq * 512:(lq + 1) * 512],
                                    start=True, stop=True)
                                nc.scalar.activation(
                                    probs[:, mt, lq * 512:(lq + 1) * 512],
                                    sp, mybir.ActivationFunctionType.Exp,
                                    bias=mask_sb[:, mt:mt + 1], scale=scale)

                    def den(lqs):
                        # denominators: one [1,512] bank per row chunk
                        for lq in lqs:
                            dpb = dpps.tile([1, 512], F32, name="dpb")
                            for mt in range(MT):
                                nc.tensor.matmul(
                                    dpb, ones_t,
                                    probs[:, mt, lq * 512:(lq + 1) * 512],
                                    start=(mt == 0), stop=(mt == MT - 1))
                            rr = rrowp.tile([1, 512], F32, name="rr")
                            rh = rrowp.tile([1, 512], F16, name="rh")
                            nc.vector.reciprocal_approx_fast(rr, dpb)
                            nc.vector.tensor_copy(rh, rr)
                            nc.sync.dma_start(out=rd_scr[h, lq, :], in_=rh)
                            nc.sync.dma_start(
                                out=rdenb[:, lq * 512:(lq + 1) * 512],
                                in_=_bcast_ap(rd_scr[h, lq, :]))

                    def attn(lqs):
                        # attention output, normalized at eviction
                        for lq in lqs:
                            ap_ = tmpps.tile([P, 512], F32, name="ap")
                            for mt in range(MT):
                                nc.tensor.matmul(
                                    ap_, vmd[:, mt, h * P:(h + 1) * P],
                                    probs[:, mt, lq * 512:(lq + 1) * 512],
                                    start=(mt == 0), stop=(mt == MT - 1))
                            c = slice(lq * 512, (lq + 1) * 512)
                            nc.vector.tensor_mul(attn_all[:, h, c], ap_,
                                                 rdenb[:, c])

                    if not last:
                        scores(range(LQ))
                        # next head's Q proj hides the exp/recip latency
                        qh_cur = qproj(h + 1)
                        if h == 3:
                            # D group-0 weights: issued here so they queue
                            # behind the first wq groups, not in front
                            nc.sync.dma_start(out=wo0, in_=woT[0])
                            nc.sync.dma_start(out=wg0, in_=wgT[0])
                        den(range(LQ))
                        attn(range(LQ))
                    else:
                        # no next head to hide behind: interleave per chunk
                        for lq in range(LQ):
                            scores([lq])
                            den([lq])
                            attn([lq])

            # ============== Phase D: O proj + gate, fused ==============
            with tc.tile_pool(name="wost", bufs=2) as wost, \
                 tc.tile_pool(name="wgst", bufs=2) as wgst, \
                 tc.tile_pool(name="gs", bufs=4) as gsp, \
                 tc.tile_pool(name="osb", bufs=1) as osbp, \
                 tc.tile_pool(name="ops", bufs=2, space="PSUM") as ops:
                for hog in range(NHG):
                    if hog == 0:
                        wo_t, wg_t = wo0, wg0
                    else:
                        wo_t = wost.tile([P, KT, 2 * P], F16, name="wo_t")
                        wg_t = wgst.tile([P, KT, 2 * P], F16, name="wg_t")
                        nc.sync.dma_start(out=wo_t, in_=woT[hog])
                        nc.sync.dma_start(out=wg_t, in_=wgT[hog])
                    for hh in range(2):
                        ho = hog * 2 + hh
                        o_sb = osbp.tile([P, R], F32, name="o_sb")
                        for lqp in range(2):
                            op2 = [ops.tile([P, 512], F32, name=f"op{j}")
                                   for j in range(2)]
                            gp2 = [ops.tile([P, 512], F32, name=f"gp{j}")
                                   for j in range(2)]
                            for kt in range(KT):
                                for j in range(2):
                                    c = slice(lqp * 1024 + j * 512,
                                              lqp * 1024 + (j + 1) * 512)
                                    nc.tensor.matmul(
                                        gp2[j],
                                        wg_t[:, kt, hh * P:(hh + 1) * P],
                                        xh[:, kt, c],
                                        start=(kt == 0), stop=(kt == KT - 1))
                            for kt in range(KT):
                                for j in range(2):
                                    c = slice(lqp * 1024 + j * 512,
                                              lqp * 1024 + (j + 1) * 512)
                                    nc.tensor.matmul(
                                        op2[j],
                                        wo_t[:, kt, hh * P:(hh + 1) * P],
                                        attn_all[:, kt, c],
                                        start=(kt == 0), stop=(kt == KT - 1))
                            for j in range(2):
                                gs = gsp.tile([P, 512], F16, name="gs")
                                nc.scalar.activation(
                                    gs, gp2[j],
                                    mybir.ActivationFunctionType.Sigmoid)
                                c = slice(lqp * 1024 + j * 512,
                                          lqp * 1024 + (j + 1) * 512)
                                nc.vector.tensor_mul(o_sb[:, c], op2[j], gs)
                        oeng = nc.sync if ho == NH - 1 else nc.gpsimd
                        oeng.dma_start(
                            out=outT[ho * P:(ho + 1) * P, :], in_=o_sb)

    nc.compile()
    return nc


import numpy as np

_H, _NH, _HD, _M = 2048, 16, 128, 256
_B, _L = 4, 4096
_RPC = 2048          # rows per core
_NCORES = 8
_EPS = 1e-6

_nc_cache = [None]


def _tile_w(wT, width):
    """[in, out] f32 -> [n, P, KT, width] fp16, p-major for large DMA lines."""
    KT = wT.shape[0] // 128
    n = wT.shape[1] // width
    return np.ascontiguousarray(
        wT.reshape(KT, 128, n, width).transpose(2, 1, 0, 3).astype(np.float16))


def _prep_core(hs_slice, mem_b, mask_b, shared):
    inp = dict(shared)
    xt = hs_slice.T.astype(np.float16)          # [H, R]
    inp["xhT"] = np.ascontiguousarray(
        xt.reshape(_H // 128, 128, -1).transpose(1, 0, 2))
    memt = mem_b.T.astype(np.float16)          # [H, M]
    inp["memT"] = np.ascontiguousarray(
        memt.reshape(_H // 128, 128, _M).transpose(1, 0, 2))
    maskb = np.where(mask_b, -LN256, -50.0).astype(np.float32)
    inp["maskb"] = np.ascontiguousarray(maskb.reshape(_M // 128, 128).T)
    return inp


def kernel(hidden_states, memory_tokens, memory_mask, norm_w,
           wq, wk, wv, wo, wg):
    import concourse.bacc as bacc

    hs = np.asarray(hidden_states, dtype=np.float32)
    mem = np.asarray(memory_tokens, dtype=np.float32)
    mask = np.asarray(memory_mask)
    norm_w = np.asarray(norm_w, dtype=np.float32)

    wq_n = (np.asarray(wq, dtype=np.float32) * norm_w[None, :]).T
    wg_n = (np.asarray(wg, dtype=np.float32) * norm_w[None, :]).T
    shared = {
        "wqT": _tile_w(np.ascontiguousarray(wq_n), 256),
        "wgT": _tile_w(np.ascontiguousarray(wg_n), 256),
        "woT": _tile_w(np.ascontiguousarray(np.asarray(wo, dtype=np.float32).T), 256),
        "wkT": _tile_w(np.ascontiguousarray(np.asarray(wk, dtype=np.float32).T), 1024),
        "wvT": _tile_w(np.ascontiguousarray(np.asarray(wv, dtype=np.float32).T), 512),
    }

    in_maps = []
    for c in range(_NCORES):
        b, half = c // 2, c % 2
        hs_slice = hs[b, half * _RPC:(half + 1) * _RPC, :]
        in_maps.append(_prep_core(hs_slice, mem[b], mask[b], shared))

    if _nc_cache[0] is None:
        nc = bacc.Bacc(None, target_bir_lowering=False, debug=False)
        build(nc, _H, _NH, _RPC, _M, eps=_EPS)
        _nc_cache[0] = nc
    nc = _nc_cache[0]

    import os
    trace = os.environ.get("KERNEL_TRACE") == "1"
    res = run_bass_kernel_spmd(nc, in_maps, core_ids=list(range(_NCORES)),
                               trace=trace)
    kernel.last_result = res

    out = np.empty((_B, _L, _H), dtype=np.float32)
    for c in range(_NCORES):
        b, half = c // 2, c % 2
        out[b, half * _RPC:(half + 1) * _RPC, :] = res.results[c]["outT"].T
    return out
